# revision 5
# baseline (speedup 1.0000x reference)
"""Trainium2 Bass kernel for nn_LowRankLinear (y = x @ (U@V).T + bias).

Strategy:
  - Data-parallel: shard the 8192 tokens across 8 NeuronCores (1024 each).
  - Low-rank factorization on-device: t.T = (V @ x.T)  [rank x tok], then
    y = t.T.T @ U.T + bias — 34 GFLOP total instead of 283 GFLOP for the
    materialized-W reference.
  - All matmul operands are laid out on host so that the contraction dim is
    the partition dim (x.T, V.T, U.T) — every DMA is a natural strided load,
    no on-device transposes.
  - bias is broadcast across partitions once via a K=1 ones-matmul, then
    fused into the PSUM->SBUF eviction as a DVE tensor_add.

Self-contained: hardcodes shapes from the problem spec; only needs the
concourse repo at /opt/trn_rl_repo (container-provided).
"""

import sys

if "/opt/trn_rl_repo" not in sys.path:
    sys.path.insert(0, "/opt/trn_rl_repo")

import numpy as np

import concourse.mybir as mybir
import concourse.tile as tile
from concourse import bacc
from concourse.bass_utils import run_bass_kernel_spmd

# Problem shapes (hardcoded per contract)
TOKENS = 8192
IN_F = 4096
OUT_F = 4096
RANK = 256
N_CORES = 8
TPC = TOKENS // N_CORES  # tokens per core = 1024

P = 128  # partitions
NG = 512  # moving free-dim per matmul (fp32 max, = 1 PSUM bank)
KC = IN_F // P  # 32 k-chunks for matmul1
RC = RANK // P  # 2 rank chunks
G = TPC // NG  # 2 token groups per core
MT = NG // P  # 4 token tiles per group
OGN = OUT_F // NG  # 8 out_f groups

F32 = mybir.dt.float32
MMDT = mybir.dt.float32r  # full-speed fp32 matmul mode (bit-identical to
# the fp32 2-pass path on TRN2 hw, verified empirically)

_CACHE = {}


def _build(mmdt):
    nc = bacc.Bacc(
        trn_type="TRN2", target_bir_lowering=False, debug=False, num_devices=N_CORES
    )
    xT = nc.dram_tensor("xT", [IN_F, TPC], mmdt, kind="ExternalInput")
    VT = nc.dram_tensor("VT", [IN_F, RANK], mmdt, kind="ExternalInput")
    UT = nc.dram_tensor("UT", [RANK, OUT_F], mmdt, kind="ExternalInput")
    bias = nc.dram_tensor("bias", [1, OUT_F], mmdt, kind="ExternalInput")
    y = nc.dram_tensor("y", [TPC, OUT_F], F32, kind="ExternalOutput")

    with tile.TileContext(nc) as tc:
        with (
            tc.tile_pool(name="const", bufs=1) as cp,
            tc.tile_pool(name="xp", bufs=6) as xp,
            tc.tile_pool(name="yp", bufs=6) as yp,
            tc.tile_pool(name="pt", bufs=4, space="PSUM") as ptp,
            tc.tile_pool(name="py", bufs=4, space="PSUM") as pyp,
        ):
            # ---- resident tensors ----
            vsb = cp.tile([P, KC * RANK], mmdt)  # V.T chunks [128, 256] x 32
            usb = cp.tile([P, RC * OUT_F], mmdt)  # U.T chunks [128, 4096] x 2
            tT = cp.tile([P, RC * TPC], mmdt)  # t.T  [rank-tile, tokens] x 2
            bb = cp.tile([P, OUT_F], F32)  # bias broadcast across partitions
            bsb = cp.tile([1, OUT_F], mmdt)
            ones = cp.tile([1, P], mmdt)
            ones_f = cp.tile([1, P], F32)

            for c in range(KC):
                nc.sync.dma_start(
                    vsb[:, c * RANK : (c + 1) * RANK], VT[c * P : (c + 1) * P, :]
                )
            for r in range(RC):
                nc.sync.dma_start(
                    usb[:, r * OUT_F : (r + 1) * OUT_F], UT[r * P : (r + 1) * P, :]
                )
            nc.sync.dma_start(bsb[:], bias[:])
            nc.gpsimd.memset(ones_f[:], 1.0)
            nc.vector.tensor_copy(ones[:], ones_f[:])

            # ---- bias broadcast: ones.T @ bias_row -> [128, NG] per group ----
            for og in range(OGN):
                pb = pyp.tile([P, NG], F32, tag="py")
                nc.tensor.matmul(
                    pb[:], ones[:], bsb[:, og * NG : (og + 1) * NG], start=True, stop=True
                )
                nc.vector.tensor_copy(bb[:, og * NG : (og + 1) * NG], pb[:])

            for g in range(G):
                # ---- matmul1: t.T[:, g] = sum_c V.T_c.T @ x.T_c ----
                pt = [
                    ptp.tile([P, NG], F32, name=f"pt{g}_{r}", tag="pt")
                    for r in range(RC)
                ]
                for c in range(KC):
                    xt = xp.tile([P, NG], mmdt)
                    nc.sync.dma_start(
                        xt[:], xT[c * P : (c + 1) * P, g * NG : (g + 1) * NG]
                    )
                    for r in range(RC):
                        nc.tensor.matmul(
                            pt[r][:],
                            vsb[:, c * RANK + r * P : c * RANK + (r + 1) * P],
                            xt[:],
                            start=(c == 0),
                            stop=(c == KC - 1),
                        )
                for r in range(RC):
                    # f32 PSUM -> f32r SBUF rounding copy
                    nc.vector.tensor_copy(
                        tT[:, r * TPC + g * NG : r * TPC + (g + 1) * NG], pt[r][:]
                    )

                # ---- matmul2: y[g*NG+m*P : .., og] = t.T_m.T @ U.T_og + bias ----
                for m in range(MT):
                    for og in range(OGN):
                        pyt = pyp.tile([P, NG], F32, tag="py")
                        for r in range(RC):
                            nc.tensor.matmul(
                                pyt[:],
                                tT[
                                    :,
                                    r * TPC + g * NG + m * P : r * TPC
                                    + g * NG
                                    + (m + 1) * P,
                                ],
                                usb[:, r * OUT_F + og * NG : r * OUT_F + (og + 1) * NG],
                                start=(r == 0),
                                stop=(r == RC - 1),
                            )
                        ysb = yp.tile([P, NG], F32)
                        nc.vector.tensor_add(
                            ysb[:], pyt[:], bb[:, og * NG : (og + 1) * NG]
                        )
                        nc.sync.dma_start(
                            y[
                                g * NG + m * P : g * NG + (m + 1) * P,
                                og * NG : (og + 1) * NG,
                            ],
                            ysb[:],
                        )
    nc.compile()
    return nc


def _get_nc():
    key = MMDT
    if key not in _CACHE:
        _CACHE[key] = _build(key)
    return _CACHE[key]


def _prep_in_maps(x, U, V, bias):
    x = np.ascontiguousarray(x, dtype=np.float32)
    VT = np.ascontiguousarray(np.asarray(V, dtype=np.float32).T)
    UT = np.ascontiguousarray(np.asarray(U, dtype=np.float32).T)
    b = np.ascontiguousarray(np.asarray(bias, dtype=np.float32).reshape(1, OUT_F))
    in_maps = []
    for i in range(N_CORES):
        xTi = np.ascontiguousarray(x[i * TPC : (i + 1) * TPC, :].T)
        in_maps.append({"xT": xTi, "VT": VT, "UT": UT, "bias": b})
    return in_maps


def kernel(x, U, V, bias):
    nc = _get_nc()
    in_maps = _prep_in_maps(x, U, V, bias)
    res = run_bass_kernel_spmd(nc, in_maps, core_ids=list(range(N_CORES)))
    return np.concatenate([res.results[i]["y"] for i in range(N_CORES)], axis=0)


def run_profiled(x, U, V, bias, **trace_kwargs):
    """Like kernel() but with NTFF tracing; returns (y, BassKernelResults)."""
    nc = _get_nc()
    in_maps = _prep_in_maps(x, U, V, bias)
    res = run_bass_kernel_spmd(
        nc, in_maps, core_ids=list(range(N_CORES)), trace=True, **trace_kwargs
    )
    y = np.concatenate([res.results[i]["y"] for i in range(N_CORES)], axis=0)
    return y, res


# revision 6
# speedup vs baseline: 1.2725x; 1.2725x over previous
"""Trainium2 Bass kernel for nn_LowRankLinear (y = x @ (U@V).T + bias).

Strategy:
  - Data-parallel: shard the 8192 tokens across 8 NeuronCores (1024 each).
  - Low-rank factorization on-device: t.T = (V @ x.T)  [rank x tok], then
    y = t.T.T @ U.T + bias — 34 GFLOP total instead of 283 GFLOP for the
    materialized-W reference.
  - All matmul operands are laid out on host so that the contraction dim is
    the partition dim (x.T, V.T, U.T) — every DMA is a natural strided load
    with fully contiguous per-partition lines, no on-device transposes.
  - Few, large DMAs (2 MB) to amortize HWDGE issue cost and descriptor
    overhead: x in 8 loads of [128, 4, 1024], y out in 8 stores of
    [128, 4096] (16 KB/partition contiguous).
  - bias is broadcast across partitions once via a K=1 ones-matmul, then
    fused into the PSUM->SBUF eviction as a DVE tensor_add.
  - float32r matmuls (bit-identical to the fp32 2-pass PE path on TRN2,
    verified on hw) with f32 PSUM accumulation.

Self-contained: hardcodes shapes from the problem spec; only needs the
concourse repo at /opt/trn_rl_repo (container-provided).
"""

import sys

if "/opt/trn_rl_repo" not in sys.path:
    sys.path.insert(0, "/opt/trn_rl_repo")

import numpy as np

import concourse.mybir as mybir
import concourse.tile as tile
from concourse import bacc
from concourse.bass_utils import run_bass_kernel_spmd

# Problem shapes (hardcoded per contract)
TOKENS = 8192
IN_F = 4096
OUT_F = 4096
RANK = 256
N_CORES = 8
TPC = TOKENS // N_CORES  # tokens per core = 1024

P = 128  # partitions
NG = 512  # moving free-dim per matmul (fp32 max, = 1 PSUM bank)
KC = IN_F // P  # 32 k-chunks for matmul1
RC = RANK // P  # 2 rank chunks
G = TPC // NG  # 2 halves of the token range (PSUM free-dim limit)
MT = TPC // P  # 8 token tiles per core for matmul2
OGN = OUT_F // NG  # 8 out_f groups
CB = 4  # k-chunks per x DMA (2 MB transfers)
XD = KC // CB  # 8 x DMAs

F32 = mybir.dt.float32
MMDT = mybir.dt.float32r  # full-speed fp32 matmul mode

_CACHE = {}


def _build(mmdt):
    nc = bacc.Bacc(
        trn_type="TRN2", target_bir_lowering=False, debug=False, num_devices=N_CORES
    )
    xT = nc.dram_tensor("xT", [IN_F, TPC], mmdt, kind="ExternalInput")
    VT = nc.dram_tensor("VT", [IN_F, RANK], mmdt, kind="ExternalInput")
    UT = nc.dram_tensor("UT", [RANK, OUT_F], mmdt, kind="ExternalInput")
    bias = nc.dram_tensor("bias", [1, OUT_F], mmdt, kind="ExternalInput")
    y = nc.dram_tensor("y", [TPC, OUT_F], F32, kind="ExternalOutput")

    with tile.TileContext(nc) as tc:
        with (
            tc.tile_pool(name="const", bufs=1) as cp,
            tc.tile_pool(name="xp", bufs=3) as xp,
            tc.tile_pool(name="yp", bufs=2) as yp,
            tc.tile_pool(name="pt", bufs=4, space="PSUM") as ptp,
            tc.tile_pool(name="py", bufs=4, space="PSUM") as pyp,
        ):
            # ---- resident tensors ----
            vsb = cp.tile([P, KC * RANK], mmdt)  # V.T chunks [128, 256] x 32
            usb = cp.tile([P, RC * OUT_F], mmdt)  # U.T chunks [128, 4096] x 2
            tT = cp.tile([P, RC * TPC], mmdt)  # t.T  [rank-tile, tokens] x 2
            bb = cp.tile([P, OUT_F], F32)  # bias broadcast across partitions
            bsb = cp.tile([1, OUT_F], mmdt)
            ones = cp.tile([1, P], mmdt)
            ones_f = cp.tile([1, P], F32)

            # V.T: one 4MB DMA ([128, 32, 256], 1KB lines)
            nc.sync.dma_start(
                vsb[:].rearrange("p (c m) -> p c m", c=KC),
                VT.rearrange("(c p) m -> p c m", p=P),
            )
            # U.T: two 2MB DMAs (16KB lines)
            for r in range(RC):
                nc.sync.dma_start(
                    usb[:, r * OUT_F : (r + 1) * OUT_F], UT[r * P : (r + 1) * P, :]
                )
            nc.sync.dma_start(bsb[:], bias[:])
            nc.gpsimd.memset(ones_f[:], 1.0)
            nc.vector.tensor_copy(ones[:], ones_f[:])

            # ---- bias broadcast: ones.T @ bias_row -> [128, NG] per group ----
            for og in range(OGN):
                pb = pyp.tile([P, NG], F32, tag="py")
                nc.tensor.matmul(
                    pb[:],
                    ones[:],
                    bsb[:, og * NG : (og + 1) * NG],
                    start=True,
                    stop=True,
                )
                nc.vector.tensor_copy(bb[:, og * NG : (og + 1) * NG], pb[:])

            # ---- matmul1: t.T = sum_c V.T_c.T @ x.T_c  (4 open psum groups) ----
            pt = [
                ptp.tile([P, NG], F32, name=f"pt{r}_{g}", tag="pt")
                for r in range(RC)
                for g in range(G)
            ]
            for d in range(XD):
                xt = xp.tile([P, CB, TPC], mmdt)
                nc.sync.dma_start(
                    xt[:],
                    xT[d * CB * P : (d + 1) * CB * P, :].rearrange(
                        "(c p) n -> p c n", p=P
                    ),
                )
                for cc in range(CB):
                    c = d * CB + cc
                    for r in range(RC):
                        for g in range(G):
                            nc.tensor.matmul(
                                pt[r * G + g][:],
                                vsb[:, c * RANK + r * P : c * RANK + (r + 1) * P],
                                xt[:, cc, g * NG : (g + 1) * NG],
                                start=(c == 0),
                                stop=(c == KC - 1),
                            )
            for r in range(RC):
                for g in range(G):
                    # f32 PSUM -> f32r SBUF rounding copy
                    nc.vector.tensor_copy(
                        tT[:, r * TPC + g * NG : r * TPC + (g + 1) * NG],
                        pt[r * G + g][:],
                    )

            # ---- matmul2: y[m] = t.T_m.T @ U.T + bias, full 16KB-line stores ----
            for m in range(MT):
                ysb = yp.tile([P, OUT_F], F32)
                for og in range(OGN):
                    pyt = pyp.tile([P, NG], F32, tag="py")
                    for r in range(RC):
                        nc.tensor.matmul(
                            pyt[:],
                            tT[:, r * TPC + m * P : r * TPC + (m + 1) * P],
                            usb[:, r * OUT_F + og * NG : r * OUT_F + (og + 1) * NG],
                            start=(r == 0),
                            stop=(r == RC - 1),
                        )
                    nc.vector.tensor_add(
                        ysb[:, og * NG : (og + 1) * NG],
                        pyt[:],
                        bb[:, og * NG : (og + 1) * NG],
                    )
                nc.sync.dma_start(y[m * P : (m + 1) * P, :], ysb[:])
    nc.compile()
    return nc


def _get_nc():
    key = MMDT
    if key not in _CACHE:
        _CACHE[key] = _build(key)
    return _CACHE[key]


def _prep_in_maps(x, U, V, bias):
    x = np.ascontiguousarray(x, dtype=np.float32)
    VT = np.ascontiguousarray(np.asarray(V, dtype=np.float32).T)
    UT = np.ascontiguousarray(np.asarray(U, dtype=np.float32).T)
    b = np.ascontiguousarray(np.asarray(bias, dtype=np.float32).reshape(1, OUT_F))
    in_maps = []
    for i in range(N_CORES):
        xTi = np.ascontiguousarray(x[i * TPC : (i + 1) * TPC, :].T)
        in_maps.append({"xT": xTi, "VT": VT, "UT": UT, "bias": b})
    return in_maps


def kernel(x, U, V, bias):
    nc = _get_nc()
    in_maps = _prep_in_maps(x, U, V, bias)
    res = run_bass_kernel_spmd(nc, in_maps, core_ids=list(range(N_CORES)))
    return np.concatenate([res.results[i]["y"] for i in range(N_CORES)], axis=0)


def run_profiled(x, U, V, bias, **trace_kwargs):
    """Like kernel() but with NTFF tracing; returns (y, BassKernelResults)."""
    nc = _get_nc()
    in_maps = _prep_in_maps(x, U, V, bias)
    res = run_bass_kernel_spmd(
        nc, in_maps, core_ids=list(range(N_CORES)), trace=True, **trace_kwargs
    )
    y = np.concatenate([res.results[i]["y"] for i in range(N_CORES)], axis=0)
    return y, res
